# revision 55
# baseline (speedup 1.0000x reference)
"""Trainium2 Bass kernel for ConvFCNet (3x conv+pool -> int8-fakequant FC + LIF SNN head).

Data-parallel over 8 NeuronCores: batch 512 -> 64 samples/core, weights replicated.

v1 rework (from 156us baseline): the PE queue is kept continuously fed so the
tensor engine stays at full p-state and is the binding resource (~89us of
matmul work):
  - conv1 im2col is built on the HOST in per-chunk order (chunk m = samples
    4m..4m+3 via the block-diagonal group trick), DMAed in 16 fine-grained
    chunks so the first matmul starts at ~4us instead of 11us.
  - conv2 blocks are emitted interleaved into the conv1 chunk loop (lag 2),
    so conv2 matmuls run while conv1 pooling drains instead of after it.
  - pooling max stages use tensor_tensor(max) (2x DVE perf mode for packed
    bf16) instead of scalar_tensor_tensor (no perf mode), and PSUM tiles span
    2 banks so one Act evacuation covers 2 matmul tiles.
  - LIF layer-1 is solved analytically across the 3 timesteps straight from
    the cur1 PSUM (s1_t thresholds 2, 4/3, 8/7 on cur1), FC2 runs all 3
    timesteps in one matmul set (N=192), and FC2/FC3 weights are pre-scaled
    by 0.5 on the host so the LIF v-update is a single scalar_tensor_tensor.
"""

import numpy as np
import ml_dtypes

import concourse.bass as bass
import concourse.bacc as bacc
import concourse.tile as tile
import concourse.mybir as mybir

AF = mybir.ActivationFunctionType
ALU = mybir.AluOpType
BF16 = mybir.dt.bfloat16
F32 = mybir.dt.float32

NCORES = 8
B = 64  # samples per core


def _v(ap, p0, npart, dims, off=0):
    """View into an SBUF/PSUM tile AP: partition slice [p0, p0+npart) + custom free dims."""
    pitch = ap.ap[0][0]
    return bass.AP(
        tensor=ap.tensor,
        offset=ap.offset + p0 * pitch + off,
        ap=[[pitch, npart]] + [list(d) for d in dims],
    )


def _dv(ap, off, dims):
    """View into a DRAM tensor AP with custom dims."""
    return bass.AP(tensor=ap.tensor, offset=ap.offset + off, ap=[list(d) for d in dims])


def _emit(tc, io):
    nc = tc.nc
    from contextlib import ExitStack

    with ExitStack() as ctx:
        # ---------------- persistent buffers + weights ----------------
        # weights ride the Activation HWDGE queue: Pool stays free for buf96 descriptor
        # generation and SP for the im2col stream
        c1i = ctx.enter_context(tc.tile_pool(name="c1imc", bufs=5))
        imct0 = c1i.tile([109, 2400], BF16, tag="imc", name="imc")
        nc.sync.dma_start(
            _v(imct0, 0, 109, [[1, 800]]),
            _dv(io["imc"], 0, [[40000, 109], [1, 800]]),
        )
        nc.sync.dma_start(
            _v(imct0, 0, 109, [[1, 1600]], 800),
            _dv(io["imc"], 800, [[40000, 109], [1, 1600]]),
        )
        wp = ctx.enter_context(tc.tile_pool(name="wts", bufs=1))
        w1sb = wp.tile([109, 128], BF16)
        nc.scalar.dma_start(w1sb[:, :], io["w1l"][:, :])
        w2sb = wp.tile([97, 192], BF16)
        nc.scalar.dma_start(w2sb[:, :], io["w2l"][:, :])
        w3asb = wp.tile([128, 384], BF16)
        w3bsb = wp.tile([65, 384], BF16)
        wf2sb = wp.tile([128, 512], BF16)
        wf3sb = wp.tile([128, 5], BF16)

        # preload the Relu activation table while the head DMAs run
        scr = wp.tile([1, 8], BF16)
        nc.scalar.activation(_v(scr, 0, 1, [[1, 8]]), _v(w1sb, 0, 1, [[1, 8]]), AF.Relu)

        mp = ctx.enter_context(tc.tile_pool(name="main", bufs=1))
        # conv1 pooled output, padded 26x26; partition 32g+c = sample 4m+g at col m*676
        xpad2 = mp.tile([128, 16 * 676 + 4], BF16)
        for dims, off in [
            ([[676, 16], [1, 26]], 0),        # top row
            ([[676, 16], [1, 26]], 650),      # bottom row
            ([[676, 16], [26, 26]], 0),       # left col
            ([[676, 16], [26, 26]], 25),      # right col
            ([[1, 4]], 16 * 676),             # tail pad (im2col dx over-read)
        ]:
            nc.gpsimd.memset(_v(xpad2, 0, 128, dims, off), 0.0)
        # conv2 pooled output, padded 14x14, partition 64h+c holds samples of parity h
        xpad3 = mp.tile([128, 32 * 198 + 4], BF16)
        # conv3 pooled output (features): [128c, sample*36 + hw]
        feat = mp.tile([128, B * 36], BF16)

        # LIF state
        lifp = ctx.enter_context(tc.tile_pool(name="lif", bufs=1))
        zeros = lifp.tile([128, 64], F32)
        v2 = lifp.tile([128, 64], F32)
        v3 = lifp.tile([5, 64], F32)
        acc = lifp.tile([5, 64], F32)
        s1_all = lifp.tile([128, 768], BF16)   # [t*256 + cur1-col]
        s2_all = lifp.tile([128, 192], BF16)   # [t*64 + sample-col]

        # conv3 im2col buffers (row 64 of B = bias row)
        c3b = ctx.enter_context(tc.tile_pool(name="c3buf", bufs=1))
        bufA = [c3b.tile([128, 32 * 198 + 4], BF16, name=f"bufA{h}") for h in range(2)]
        bufB = [c3b.tile([65, 32 * 198 + 4], BF16, name=f"bufB{h}") for h in range(2)]

        def late_inits():
            # not needed until conv2/the tail: emitted on the gpsimd queue after
            # the first buf96 pair DMAs so they don't delay the conv2 start
            for dims, off in [
                ([[198, 32], [1, 14]], 0),        # top row
                ([[198, 32], [1, 14]], 182),      # bottom row
                ([[198, 32], [14, 14]], 0),       # left col
                ([[198, 32], [14, 14]], 13),      # right col
                ([[1, 4]], 32 * 198),             # tail pad (im2col dx over-read)
                ([[198, 32], [1, 2]], 196),       # per-sample slack (pitch 198 vs 196)
            ]:
                nc.gpsimd.memset(_v(xpad3, 0, 128, dims, off), 0.0)
            for t in (zeros, v2, v3, acc):
                nc.gpsimd.memset(t[:, :], 0.0)
            nc.gpsimd.dma_start(w3asb[:, :], io["w3a"][:, :])
            nc.gpsimd.dma_start(w3bsb[:, :], io["w3b"][:, :])
            nc.gpsimd.dma_start(wf2sb[:, :], io["wf2"][:, :])
            nc.gpsimd.dma_start(wf3sb[:, :], io["wf3"][:, :])
            for h in range(2):
                nc.gpsimd.dma_start(_v(bufB[h], 64, 1, [[1, 32 * 198 + 4]]), io["ones"][0:1, 0 : 32 * 198 + 4])

        # FC1 weights: loaded in 4 chunks spread across the conv1/conv2 window
        # (a single 13us DMA would block the serialized DMA engines)
        fcw = ctx.enter_context(tc.tile_pool(name="fcw", bufs=1))
        wf1sb = fcw.tile([128, 18432], BF16)

        # conv2 im2col quarters (96 rows = 32c x 3dx, row 96 = bias row), scoped
        b96 = ctx.enter_context(tc.tile_pool(name="b96", bufs=2))
        bqs = {}

        # ---------------- conv1 + conv2 + conv3 (interleaved, PE stays fed) ----------------
        with (
            tc.tile_pool(name="c2ps", bufs=4, space="PSUM") as c2p,
            tc.tile_pool(name="c2t", bufs=4) as c2t,
        ):
            imcts = {0: imct0}

            def imc_dma(m):
                imct = c1i.tile([109, 2400], BF16, tag="imc", name="imc")
                nc.sync.dma_start(
                    _v(imct, 0, 109, [[1, 2400]]),
                    _dv(io["imc"], m * 2500, [[40000, 109], [1, 2400]]),
                )
                imcts[m] = imct

            def conv1_chunk(m, c1p, c1t, yts=range(6)):
                imct = imcts[m]
                base = m * 676 + 27
                for yt in yts:
                    ps = c1p.tile([128, 384], F32, tag="ps1", name="ps1")
                    nc.tensor.matmul(
                        ps[:, :],
                        _v(w1sb, 0, 109, [[1, 128]]),
                        _v(imct, 0, 109, [[50, 8], [1, 48]], yt * 400),
                        start=True,
                        stop=True,
                    )
                    if yt < 4:
                        # Act evac: relu+copy, x-deinterleaved (y,xh,phase)
                        stg = c1t.tile([128, 384], BF16, tag="stg", name="stg")
                        nc.scalar.activation(
                            _v(stg, 0, 128, [[24, 8], [1, 24], [192, 2]]),
                            ps[:, :],
                            AF.Relu,
                        )
                        # max stages as tensor_tensor (2x DVE mode on packed bf16)
                        xm = c1t.tile([128, 192], BF16, tag="xm", name="xm")
                        nc.vector.tensor_tensor(
                            _v(xm, 0, 128, [[1, 192]]),
                            _v(stg, 0, 128, [[1, 192]]),
                            _v(stg, 0, 128, [[1, 192]], 192),
                            ALU.max,
                        )
                        nc.vector.tensor_tensor(
                            _v(xpad2, 0, 128, [[26, 4], [1, 24]], base + yt * 4 * 26),
                            _v(xm, 0, 128, [[48, 4], [1, 24]]),
                            _v(xm, 0, 128, [[48, 4], [1, 24]], 24),
                            ALU.max,
                        )
                    else:
                        # DVE: direct 2x2 max-reduce from PSUM (relu deferred)
                        nc.vector.tensor_reduce(
                            _v(xpad2, 0, 128, [[26, 4], [1, 24]], base + yt * 4 * 26),
                            _v(ps, 0, 128, [[96, 4], [2, 24], [48, 2], [1, 2]]),
                            mybir.AxisListType.XY,
                            ALU.max,
                        )
                if 5 in yts:
                    rows = _v(xpad2, 0, 128, [[26, 8], [1, 24]], base + 16 * 26)
                    nc.vector.tensor_scalar(rows, rows, 0.0, None, ALU.max)
                # conv2 im2col: batched per chunk-PAIR (8 samples) on the gpsimd
                # SWDGE queue; quarter col layout is (g, chunk): sample
                # 16Q+4c+g at col (4g+c)*676
                Q = m // 4
                if m % 4 == 0:
                    bq = b96.tile([97, 16 * 676], BF16, tag="bq", name="bq")
                    bqs[Q] = bq
                    nc.gpsimd.dma_start(_v(bq, 96, 1, [[1, 16 * 676]]), io["ones"][0:1, 0 : 16 * 676])
                if m % 2 == 1:
                    bq = bqs[Q]
                    c0 = 2 * ((m // 2) % 2)
                    for g in range(4):
                        nc.gpsimd.dma_start(
                            _v(bq, 0, 96, [[1, 1352]], (4 * g + c0) * 676),
                            _v(xpad2, 32 * g, 32, [[1, 3], [1, 1352]], (m - 1) * 676),
                        )


            def conv2_block(b):
                bq = bqs[b // 8]
                for yh in range(2):
                    ps = c2p.tile([128, 288], F32, tag="ps2", name="ps2")
                    for h in range(2):
                        s = 2 * b + h
                        loc = 4 * (s % 4) + (s // 4 - 4 * (b // 8))
                        for dy in range(3):
                            nc.tensor.matmul(
                                _v(ps, 64 * h, 64, [[1, 288]]),
                                w2sb[0:97, dy * 64 : dy * 64 + 64],
                                _v(bq, 0, 97, [[26, 12], [1, 24]], loc * 676 + yh * 312 + dy * 26),
                                start=(dy == 0),
                                stop=(dy == 2),
                                tile_position=(0, 64 * h),
                            )
                    # Act evac (y,xh,phase), then 2x tt max stages
                    stg = c2t.tile([128, 288], BF16, tag="stg", name="stg")
                    nc.scalar.activation(
                        _v(stg, 0, 128, [[12, 12], [1, 12], [144, 2]]),
                        _v(ps, 0, 128, [[24, 12], [2, 12], [1, 2]]),
                        AF.Relu,
                    )
                    xm = c2t.tile([128, 144], BF16, tag="xm", name="xm")
                    nc.vector.tensor_tensor(
                        _v(xm, 0, 128, [[1, 144]]),
                        _v(stg, 0, 128, [[1, 144]]),
                        _v(stg, 0, 128, [[1, 144]], 144),
                        ALU.max,
                    )
                    nc.vector.tensor_tensor(
                        _v(xpad3, 0, 128, [[14, 6], [1, 12]], b * 198 + 15 + yh * 84),
                        _v(xm, 0, 128, [[24, 6], [1, 12]]),
                        _v(xm, 0, 128, [[24, 6], [1, 12]], 12),
                        ALU.max,
                    )
                # conv3 im2col chunk once its xpad3 sample range is complete
                if b == 15 or b == 31:
                    ck = b // 16
                    off = ck * 16 * 198
                    for h in range(2):
                        nc.gpsimd.dma_start(
                            _v(bufA[h], 0, 128, [[1, 16 * 198]], off),
                            _v(xpad3, 64 * h, 64, [[1, 2], [1, 16 * 198]], off),
                        )
                        nc.gpsimd.dma_start(
                            _v(bufB[h], 0, 64, [[1, 16 * 198]], off),
                            _v(xpad3, 64 * h, 64, [[1, 16 * 198]], off + 2),
                        )

            def conv3_unit(h, bp):
                # c3p/c3t are opened after the conv1 PSUM pool closes (bank budget)
                bj = bp % 4
                ps = c3p.tile([128, 288], F32, tag="ps3", name="ps3")
                for dy in range(3):
                    dims = [[198, 2], [14, 12], [1, 12]]
                    off = bp * 2 * 198 + dy * 14
                    nc.tensor.matmul(
                        ps[:, :], w3asb[0:128, dy * 128 : dy * 128 + 128],
                        _v(bufA[h], 0, 128, dims, off),
                        start=(dy == 0), stop=False,
                    )
                    nc.tensor.matmul(
                        ps[:, :], w3bsb[0:65, dy * 128 : dy * 128 + 128],
                        _v(bufB[h], 0, 65, dims, off),
                        start=False, stop=(dy == 2),
                    )
                # slot of (h, bp, i) is sample 4bp+h+2i -> feat col (4bp+h+2i)*36
                if bj < 3:
                    stg = c3t.tile([128, 288], BF16, tag="stg", name="stg")
                    nc.scalar.activation(
                        _v(stg, 0, 128, [[72, 2], [6, 12], [1, 6], [144, 2]]),
                        ps[:, :], AF.Relu,
                    )
                    xm = c3t.tile([128, 144], BF16, tag="xm", name="xm")
                    nc.vector.tensor_tensor(
                        _v(xm, 0, 128, [[1, 144]]),
                        _v(stg, 0, 128, [[1, 144]]),
                        _v(stg, 0, 128, [[1, 144]], 144),
                        ALU.max,
                    )
                    nc.vector.tensor_tensor(
                        _v(feat, 0, 128, [[72, 2], [6, 6], [1, 6]], (4 * bp + h) * 36),
                        _v(xm, 0, 128, [[72, 2], [12, 6], [1, 6]]),
                        _v(xm, 0, 128, [[72, 2], [12, 6], [1, 6]], 6),
                        ALU.max,
                    )
                else:
                    # DVE direct reduce per sample (relu deferred to feat pass)
                    for i in range(2):
                        nc.vector.tensor_reduce(
                            _v(feat, 0, 128, [[6, 6], [1, 6]], (4 * bp + h + 2 * i) * 36),
                            _v(ps, 0, 128, [[24, 6], [2, 6], [12, 2], [1, 2]], i * 144),
                            mybir.AxisListType.XY,
                            ALU.max,
                        )

            with (
                tc.tile_pool(name="c1ps", bufs=4, space="PSUM") as c1p,
                tc.tile_pool(name="c1t", bufs=4) as c1t,
            ):
                for m in range(16):
                    if m + 1 < 16:
                        imc_dma(m + 1)
                    conv1_chunk(m, c1p, c1t)
                    if m == 2:
                        late_inits()
                    if m >= 2:
                        conv2_block(2 * (m - 2))
                        conv2_block(2 * (m - 2) + 1)
            for b in range(28, 32):
                conv2_block(b)

        # ---------------- conv3 + FC1 (parity-pipelined) ----------------
        c3p = ctx.enter_context(tc.tile_pool(name="c3ps", bufs=5, space="PSUM"))
        c3t = ctx.enter_context(tc.tile_pool(name="c3t", bufs=5))
        cur1p = ctx.enter_context(tc.tile_pool(name="cur1p", bufs=1, space="PSUM"))
        cur1 = cur1p.tile([128, 256], F32)
        # ck0 units first (their im2col chunk landed at b=15); ck1's chunk
        # (emitted at b=31) and the FC1 weights transfer while these run
        for bp in range(8):
            for h in range(2):
                conv3_unit(h, bp)
            nc.gpsimd.dma_start(
                wf1sb[:, bp * 2304 : (bp + 1) * 2304],
                _dv(io["wf1"], bp * 2304, [[18432, 128], [1, 2304]]),
            )
        # s1_t straight from cur1: v=(v+c)/2, th=1, hard reset =>
        # s1_t1 = [c>=2]; s1_t2 = [c>=4/3]; s1_t3 = [c>=8/7] - [c>=4/3] + [c>=2]
        c43 = float(np.float32(4.0) / np.float32(3.0))
        c87 = float(np.float32(8.0) / np.float32(7.0))
        t87 = lifp.tile([128, 256], BF16)

        def rows_relu(h):
            # in-place relu over the DVE-reduced feat slots of this parity
            # (bp = 3,7,11,15 -> slots 4bp+h and 4bp+2+h)
            rows = _v(feat, 0, 128, [[576, 4], [72, 2], [1, 36]], (12 + h) * 36)
            nc.vector.tensor_scalar(rows, rows, 0.0, None, ALU.max)

        def fc1(h, per_g=False):
            # FC1 for parity h: out [unit, 32 samples] at cur1 col 64g+32h
            # (samples of parity h = feat cols h, h+2, ... -> stride 72)
            for g in range(4):
                for k in range(36):
                    nc.tensor.matmul(
                        cur1[:, 64 * g + 32 * h : 64 * g + 32 * h + 32],
                        wf1sb[:, k * 512 + g * 128 : k * 512 + g * 128 + 128],
                        _v(feat, 0, 128, [[72, 32]], k + 36 * h),
                        start=(k == 0),
                        stop=(k == 35),
                    )
                if per_g:
                    thr_g(h, g)

        def thresholds(h):
            # LIF layer-1 thresholds for this parity's cur1 columns
            cslc = _v(cur1, 0, 128, [[64, 4], [1, 32]], 32 * h)
            s1t = lambda t: _v(s1_all, 0, 128, [[64, 4], [1, 32]], t * 256 + 32 * h)
            nc.vector.tensor_scalar(s1t(0), cslc, 2.0, None, ALU.is_ge)
            nc.vector.tensor_scalar(s1t(1), cslc, c43, None, ALU.is_ge)
            t87s = _v(t87, 0, 128, [[64, 4], [1, 32]], 32 * h)
            nc.vector.tensor_scalar(t87s, cslc, c87, None, ALU.is_ge)
            nc.vector.tensor_tensor(s1t(2), t87s, s1t(1), ALU.subtract)
            nc.vector.tensor_tensor(s1t(2), s1t(2), s1t(0), ALU.add)

        cur2p = ctx.enter_context(tc.tile_pool(name="cur2p", bufs=1, space="PSUM"))
        dtp = ctx.enter_context(tc.tile_pool(name="liftmp", bufs=2))
        cur2 = cur2p.tile([128, 192], F32, tag="cur2")
        cur3 = cur2p.tile([5, 192], F32, tag="cur3")

        def thr_g(h, g):
            # layer-1 thresholds for one (parity, unit-group) block of cur1
            cslc = _v(cur1, 0, 128, [[1, 32]], 64 * g + 32 * h)
            s1t = lambda t: _v(s1_all, 0, 128, [[1, 32]], t * 256 + 64 * g + 32 * h)
            t87s = _v(t87, 0, 128, [[1, 32]], 64 * g + 32 * h)
            nc.vector.tensor_scalar(s1t(0), cslc, 2.0, None, ALU.is_ge)
            nc.vector.tensor_scalar(s1t(1), cslc, c43, None, ALU.is_ge)
            nc.vector.tensor_scalar(t87s, cslc, c87, None, ALU.is_ge)
            nc.vector.tensor_tensor(s1t(2), t87s, s1t(1), ALU.subtract)
            nc.vector.tensor_tensor(s1t(2), s1t(2), s1t(0), ALU.add)

        def fc2(h, ts=(0, 1, 2)):
            # FC2 for one parity over the given timesteps (wf2 pre-scaled 0.5)
            nt = len(ts)
            t0 = ts[0]
            for g in range(4):
                nc.tensor.matmul(
                    _v(cur2, 0, 128, [[64, nt], [1, 32]], 64 * t0 + 32 * h),
                    wf2sb[:, g * 128 : g * 128 + 128],
                    _v(s1_all, 0, 128, [[256, nt], [1, 32]], 256 * t0 + 64 * g + 32 * h),
                    start=(g == 0), stop=(g == 3),
                )

        def lif2(h, t):
            # v2 <- v2*0.5 + cur2_half ; s2 = (v2 >= 1) ; v2 <- 0 where s2 ; FC3
            v2h = v2[:, 32 * h : 32 * h + 32]
            s2t = _v(s2_all, 0, 128, [[1, 32]], 64 * t + 32 * h)
            nc.vector.scalar_tensor_tensor(
                v2h, v2h, 0.5, cur2[:, 64 * t + 32 * h : 64 * t + 32 * h + 32], ALU.mult, ALU.add)
            nc.vector.tensor_scalar(s2t, v2h, 1.0, None, ALU.is_ge)
            nc.vector.copy_predicated(v2h, s2t.bitcast(mybir.dt.uint16), zeros[:, 0:32])
            nc.tensor.matmul(
                cur3[0:5, 64 * t + 32 * h : 64 * t + 32 * h + 32], wf3sb[0:128, 0:5],
                _v(s2_all, 0, 128, [[1, 32]], 64 * t + 32 * h),
                start=True, stop=True,
            )

        def lif3(h, t):
            v3h = v3[0:5, 32 * h : 32 * h + 32]
            nc.vector.scalar_tensor_tensor(
                v3h, v3h, 0.5, cur3[0:5, 64 * t + 32 * h : 64 * t + 32 * h + 32], ALU.mult, ALU.add)
            s3 = dtp.tile([5, 32], F32, tag="s3")
            nc.vector.tensor_scalar(s3[:, :], v3h, 1.0, None, ALU.is_ge)
            nc.vector.copy_predicated(v3h, s3[:, :].bitcast(mybir.dt.uint32), zeros[0:5, 0:32])
            nc.vector.tensor_tensor(
                acc[0:5, 32 * h : 32 * h + 32], acc[0:5, 32 * h : 32 * h + 32], s3[:, :], ALU.add)

        def finish(h):
            # acc/3 for acc in {0,1,2,3}: mult by fp32(1/3) matches true division
            # except acc=3 (3*0.33333334 = 1.0000001) -> clamp with min(., 1.0).
            acch = acc[0:5, 32 * h : 32 * h + 32]
            nc.vector.tensor_scalar(acch, acch, float(np.float32(1.0) / np.float32(3.0)), 1.0, ALU.mult, ALU.min)
            # acc col (32h + j) holds sample 2j+h -> un-permute on the way out
            nc.sync.dma_start(
                _dv(io["out"], h, [[64, 5], [2, 32]]),
                _v(acc, 0, 5, [[1, 32]], 32 * h),
            )

        def lif_tail(h):
            fc2(h)
            lif2(h, 0)
            lif2(h, 1)
            lif3(h, 0)
            lif2(h, 2)
            lif3(h, 1)
            lif3(h, 2)

        # parity-0 ck1 units, then FC1(0) hidden behind the first parity-1
        # units; parity-0's whole LIF chain runs under conv3(1)/FC1(1)
        for bp in range(8, 16):
            conv3_unit(0, bp)
        rows_relu(0)
        for bp in range(8, 11):
            conv3_unit(1, bp)
        fc1(0)
        thresholds(0)
        fc2(0)
        conv3_unit(1, 11)
        conv3_unit(1, 12)
        lif2(0, 0)
        lif2(0, 1)
        lif2(0, 2)
        conv3_unit(1, 13)
        conv3_unit(1, 14)
        conv3_unit(1, 15)
        lif3(0, 0)
        lif3(0, 1)
        lif3(0, 2)
        finish(0)
        rows_relu(1)
        fc1(1)
        cslc1 = _v(cur1, 0, 128, [[64, 4], [1, 32]], 32)
        s1s = lambda t: _v(s1_all, 0, 128, [[64, 4], [1, 32]], t * 256 + 32)
        nc.vector.tensor_scalar(s1s(0), cslc1, 2.0, None, ALU.is_ge)
        nc.vector.tensor_scalar(s1s(1), cslc1, c43, None, ALU.is_ge)
        fc2(1, ts=(0, 1))
        t87s1 = _v(t87, 0, 128, [[64, 4], [1, 32]], 32)
        nc.vector.tensor_scalar(t87s1, cslc1, c87, None, ALU.is_ge)
        nc.vector.tensor_tensor(s1s(2), t87s1, s1s(1), ALU.subtract)
        nc.vector.tensor_tensor(s1s(2), s1s(2), s1s(0), ALU.add)
        fc2(1, ts=(2,))
        lif2(1, 0)
        lif2(1, 1)
        lif3(1, 0)
        lif2(1, 2)
        lif3(1, 1)
        lif3(1, 2)
        finish(1)


def _build():
    nc = bacc.Bacc("TRN2", target_bir_lowering=False, debug=False, enable_asserts=True)
    io = {}

    def inp(name, shape, dt):
        io[name] = nc.dram_tensor(name, shape, dt, kind="ExternalInput").ap()

    inp("imc", [109, 40000], BF16)
    inp("w1l", [109, 128], BF16)
    inp("w2l", [97, 192], BF16)
    inp("w3a", [128, 384], BF16)
    inp("w3b", [65, 384], BF16)
    inp("wf1", [128, 18432], BF16)
    inp("wf2", [128, 512], BF16)
    inp("wf3", [128, 5], BF16)
    inp("ones", [1, 10816], BF16)
    io["out"] = nc.dram_tensor("out", [5, 64], F32, kind="ExternalOutput").ap()

    with tile.TileContext(nc) as tc:
        _emit(tc, io)
    nc.compile()
    return nc


def _fake_quant(w):
    w = np.asarray(w, np.float32)
    scale = np.float32(np.max(np.abs(w)) / np.float32(127.0))
    wq = np.clip(np.round(w / scale), -127.0, 127.0).astype(np.float32) * scale
    return wq.astype(np.float32)


def _bf16(a):
    return np.asarray(a, np.float32).astype(ml_dtypes.bfloat16)


def _prep_weights(conv1_w, conv1_b, conv2_w, conv2_b, conv3_w, conv3_b, W1, W2, W3):
    c1 = np.asarray(conv1_w, np.float32)  # [32, 3, 3, 3]
    c2 = np.asarray(conv2_w, np.float32)  # [64, 32, 3, 3]
    c3 = np.asarray(conv3_w, np.float32)  # [128, 64, 3, 3]

    # conv1 block-diagonal: rows 27g..27g+26 = taps of group g -> cols 32g..32g+31;
    # row 108 = bias (tiled 4x over the 4 col groups).
    w1l = np.zeros((109, 128), np.float32)
    wk = c1.transpose(2, 3, 1, 0).reshape(27, 32)  # [(dy,dx,c), m]
    for q in range(4):
        w1l[27 * q : 27 * q + 27, 32 * q : 32 * q + 32] = wk
    w1l[108, :] = np.tile(np.asarray(conv1_b, np.float32), 4)

    w2l = np.zeros((97, 192), np.float32)
    w2l[0:96] = c2.transpose(1, 3, 2, 0).reshape(96, 192)  # [(c,dx), (dy,m)]
    w2l[96, 0:64] = np.asarray(conv2_b, np.float32)        # bias rides the dy=0 block

    w3x = c3.transpose(1, 3, 2, 0)  # [c, dx, dy, m]
    w3a = w3x[:, 0:2].reshape(128, 384)
    w3b = np.zeros((65, 384), np.float32)
    w3b[0:64] = w3x[:, 2].reshape(64, 384)
    w3b[64, 0:128] = np.asarray(conv3_b, np.float32)       # bias rides the dy=0 block

    W1q = _fake_quant(W1)  # [512, 4608]
    W2q = _fake_quant(W2)  # [128, 512]
    W3q = _fake_quant(W3)  # [5, 128]

    # [c, k*512 + u] = W1q[u, c*36 + k]  (FC1 weight-stationary: out [unit, sample])
    wf1 = W1q.reshape(512, 128, 36).transpose(1, 2, 0).reshape(128, 36 * 512)
    # FC2/FC3 pre-scaled by 0.5: LIF v-update becomes v*0.5 + cur_half in one op
    wf2 = 0.5 * W2q.T.reshape(4, 128, 128).transpose(1, 0, 2).reshape(128, 512)
    wf3 = 0.5 * W3q.T  # [128, 5]

    return {
        "w1l": _bf16(w1l),
        "w2l": _bf16(w2l),
        "w3a": _bf16(w3a),
        "w3b": _bf16(w3b),
        "wf1": _bf16(wf1),
        "wf2": _bf16(wf2),
        "wf3": _bf16(wf3),
        "ones": _bf16(np.ones((1, 10816), np.float32)),
    }


_NC = None
LAST_RESULTS = None


def kernel(x, conv1_w, conv1_b, conv2_w, conv2_b, conv3_w, conv3_b, W1, W2, W3, _trace=False):
    global _NC, LAST_RESULTS
    if _NC is None:
        _NC = _build()

    wmap = _prep_weights(conv1_w, conv1_b, conv2_w, conv2_b, conv3_w, conv3_b, W1, W2, W3)

    # host-side im2col for conv1, chunk-ordered: chunk m = samples 4m..4m+3,
    # imc[27g + dy*9 + dx*3 + c, m*2500 + p] = xpad[4m+g, c, p + dy*50 + dx]
    x = np.asarray(x, np.float32)
    xp = np.zeros((512, 3, 50, 50), np.float32)
    xp[:, :, 1:49, 1:49] = x
    xf = np.zeros((512, 3, 2604), np.float32)
    xf[:, :, :2500] = xp.reshape(512, 3, 2500)

    in_maps = []
    for i in range(NCORES):
        S = xf[B * i : B * (i + 1)].reshape(16, 4, 3, 2604)
        A = np.zeros((109, 40000), np.float32)
        for dy in range(3):
            for dx in range(3):
                blk = S[:, :, :, dy * 50 + dx : dy * 50 + dx + 2500]  # [m, g, c, p]
                for g in range(4):
                    r = 27 * g + dy * 9 + dx * 3
                    A[r : r + 3] = blk[:, g].transpose(1, 0, 2).reshape(3, 40000)
        A[108] = 1.0
        in_maps.append({"imc": _bf16(A), **wmap})

    from concourse.bass_utils import run_bass_kernel_spmd

    res = run_bass_kernel_spmd(_NC, in_maps, core_ids=list(range(NCORES)), trace=_trace)
    LAST_RESULTS = res
    out = np.concatenate([np.asarray(res.results[i]["out"]).T for i in range(NCORES)], axis=0)
    return np.ascontiguousarray(out.astype(np.float32))


# revision 56
# speedup vs baseline: 1.0033x; 1.0033x over previous
"""Trainium2 Bass kernel for ConvFCNet (3x conv+pool -> int8-fakequant FC + LIF SNN head).

Data-parallel over 8 NeuronCores: batch 512 -> 64 samples/core, weights replicated.

v1 rework (from 156us baseline): the PE queue is kept continuously fed so the
tensor engine stays at full p-state and is the binding resource (~89us of
matmul work):
  - conv1 im2col is built on the HOST in per-chunk order (chunk m = samples
    4m..4m+3 via the block-diagonal group trick), DMAed in 16 fine-grained
    chunks so the first matmul starts at ~4us instead of 11us.
  - conv2 blocks are emitted interleaved into the conv1 chunk loop (lag 2),
    so conv2 matmuls run while conv1 pooling drains instead of after it.
  - pooling max stages use tensor_tensor(max) (2x DVE perf mode for packed
    bf16) instead of scalar_tensor_tensor (no perf mode), and PSUM tiles span
    2 banks so one Act evacuation covers 2 matmul tiles.
  - LIF layer-1 is solved analytically across the 3 timesteps straight from
    the cur1 PSUM (s1_t thresholds 2, 4/3, 8/7 on cur1), FC2 runs all 3
    timesteps in one matmul set (N=192), and FC2/FC3 weights are pre-scaled
    by 0.5 on the host so the LIF v-update is a single scalar_tensor_tensor.
"""

import numpy as np
import ml_dtypes

import concourse.bass as bass
import concourse.bacc as bacc
import concourse.tile as tile
import concourse.mybir as mybir

AF = mybir.ActivationFunctionType
ALU = mybir.AluOpType
BF16 = mybir.dt.bfloat16
F32 = mybir.dt.float32

NCORES = 8
B = 64  # samples per core


def _v(ap, p0, npart, dims, off=0):
    """View into an SBUF/PSUM tile AP: partition slice [p0, p0+npart) + custom free dims."""
    pitch = ap.ap[0][0]
    return bass.AP(
        tensor=ap.tensor,
        offset=ap.offset + p0 * pitch + off,
        ap=[[pitch, npart]] + [list(d) for d in dims],
    )


def _dv(ap, off, dims):
    """View into a DRAM tensor AP with custom dims."""
    return bass.AP(tensor=ap.tensor, offset=ap.offset + off, ap=[list(d) for d in dims])


def _emit(tc, io):
    nc = tc.nc
    from contextlib import ExitStack

    with ExitStack() as ctx:
        # ---------------- persistent buffers + weights ----------------
        # weights ride the Activation HWDGE queue: Pool stays free for buf96 descriptor
        # generation and SP for the im2col stream
        c1i = ctx.enter_context(tc.tile_pool(name="c1imc", bufs=5))
        imct0 = c1i.tile([109, 2400], BF16, tag="imc", name="imc")
        nc.sync.dma_start(
            _v(imct0, 0, 109, [[1, 800]]),
            _dv(io["imc"], 0, [[40000, 109], [1, 800]]),
        )
        nc.sync.dma_start(
            _v(imct0, 0, 109, [[1, 1600]], 800),
            _dv(io["imc"], 800, [[40000, 109], [1, 1600]]),
        )
        wp = ctx.enter_context(tc.tile_pool(name="wts", bufs=1))
        w1sb = wp.tile([109, 128], BF16)
        nc.scalar.dma_start(w1sb[:, :], io["w1l"][:, :])
        w2sb = wp.tile([97, 192], BF16)
        nc.scalar.dma_start(w2sb[:, :], io["w2l"][:, :])
        w3asb = wp.tile([128, 384], BF16)
        w3bsb = wp.tile([65, 384], BF16)
        wf2sb = wp.tile([128, 512], BF16)
        wf3sb = wp.tile([128, 5], BF16)

        # preload the Relu activation table while the head DMAs run
        scr = wp.tile([1, 8], BF16)
        nc.scalar.activation(_v(scr, 0, 1, [[1, 8]]), _v(w1sb, 0, 1, [[1, 8]]), AF.Relu)

        mp = ctx.enter_context(tc.tile_pool(name="main", bufs=1))
        # conv1 pooled output, padded 26x26; partition 32g+c = sample 4m+g at col m*676
        xpad2 = mp.tile([128, 16 * 676 + 4], BF16)
        for dims, off in [
            ([[676, 16], [1, 26]], 0),        # top row
            ([[676, 16], [1, 26]], 650),      # bottom row
            ([[676, 16], [26, 26]], 0),       # left col
            ([[676, 16], [26, 26]], 25),      # right col
            ([[1, 4]], 16 * 676),             # tail pad (im2col dx over-read)
        ]:
            nc.gpsimd.memset(_v(xpad2, 0, 128, dims, off), 0.0)
        # conv2 pooled output, padded 14x14, partition 64h+c holds samples of parity h
        xpad3 = mp.tile([128, 32 * 198 + 4], BF16)
        # conv3 pooled output (features): [128c, sample*36 + hw]
        feat = mp.tile([128, B * 36], BF16)

        # LIF state
        lifp = ctx.enter_context(tc.tile_pool(name="lif", bufs=1))
        zeros = lifp.tile([128, 64], F32)
        v2 = lifp.tile([128, 64], F32)
        v3 = lifp.tile([5, 64], F32)
        acc = lifp.tile([5, 64], F32)
        s1_all = lifp.tile([128, 768], BF16)   # [t*256 + cur1-col]
        s2_all = lifp.tile([128, 192], BF16)   # [t*64 + sample-col]

        # conv3 im2col buffers (row 64 of B = bias row)
        c3b = ctx.enter_context(tc.tile_pool(name="c3buf", bufs=1))
        bufA = [c3b.tile([128, 32 * 198 + 4], BF16, name=f"bufA{h}") for h in range(2)]
        bufB = [c3b.tile([65, 32 * 198 + 4], BF16, name=f"bufB{h}") for h in range(2)]

        def late_inits():
            # not needed until conv2/the tail: emitted on the gpsimd queue after
            # the first buf96 pair DMAs so they don't delay the conv2 start
            for dims, off in [
                ([[198, 32], [1, 14]], 0),        # top row
                ([[198, 32], [1, 14]], 182),      # bottom row
                ([[198, 32], [14, 14]], 0),       # left col
                ([[198, 32], [14, 14]], 13),      # right col
                ([[1, 4]], 32 * 198),             # tail pad (im2col dx over-read)
                ([[198, 32], [1, 2]], 196),       # per-sample slack (pitch 198 vs 196)
            ]:
                nc.gpsimd.memset(_v(xpad3, 0, 128, dims, off), 0.0)
            for t in (zeros, v2, v3, acc):
                nc.gpsimd.memset(t[:, :], 0.0)
            nc.gpsimd.dma_start(w3asb[:, :], io["w3a"][:, :])
            nc.gpsimd.dma_start(w3bsb[:, :], io["w3b"][:, :])
            nc.gpsimd.dma_start(wf2sb[:, :], io["wf2"][:, :])
            nc.gpsimd.dma_start(wf3sb[:, :], io["wf3"][:, :])
            for h in range(2):
                nc.gpsimd.dma_start(_v(bufB[h], 64, 1, [[1, 32 * 198 + 4]]), io["ones"][0:1, 0 : 32 * 198 + 4])

        # FC1 weights: loaded in 4 chunks spread across the conv1/conv2 window
        # (a single 13us DMA would block the serialized DMA engines)
        fcw = ctx.enter_context(tc.tile_pool(name="fcw", bufs=1))
        wf1sb = fcw.tile([128, 18432], BF16)

        # conv2 im2col quarters (96 rows = 32c x 3dx, row 96 = bias row), scoped
        b96 = ctx.enter_context(tc.tile_pool(name="b96", bufs=2))
        bqs = {}

        # ---------------- conv1 + conv2 + conv3 (interleaved, PE stays fed) ----------------
        with (
            tc.tile_pool(name="c2ps", bufs=4, space="PSUM") as c2p,
            tc.tile_pool(name="c2t", bufs=3) as c2t,
        ):
            imcts = {0: imct0}

            def imc_dma(m):
                imct = c1i.tile([109, 2400], BF16, tag="imc", name="imc")
                nc.sync.dma_start(
                    _v(imct, 0, 109, [[1, 2400]]),
                    _dv(io["imc"], m * 2500, [[40000, 109], [1, 2400]]),
                )
                imcts[m] = imct

            def conv1_chunk(m, c1p, c1t, yts=(0, 4, 1, 2, 5, 3)):
                imct = imcts[m]
                base = m * 676 + 27
                for yt in yts:
                    ps = c1p.tile([128, 384], F32, tag="ps1", name="ps1")
                    nc.tensor.matmul(
                        ps[:, :],
                        _v(w1sb, 0, 109, [[1, 128]]),
                        _v(imct, 0, 109, [[50, 8], [1, 48]], yt * 400),
                        start=True,
                        stop=True,
                    )
                    if yt < 4:
                        # Act evac: relu+copy, x-deinterleaved (y,xh,phase)
                        stg = c1t.tile([128, 384], BF16, tag="stg", name="stg")
                        nc.scalar.activation(
                            _v(stg, 0, 128, [[24, 8], [1, 24], [192, 2]]),
                            ps[:, :],
                            AF.Relu,
                        )
                        # max stages as tensor_tensor (2x DVE mode on packed bf16)
                        xm = c1t.tile([128, 192], BF16, tag="xm", name="xm")
                        nc.vector.tensor_tensor(
                            _v(xm, 0, 128, [[1, 192]]),
                            _v(stg, 0, 128, [[1, 192]]),
                            _v(stg, 0, 128, [[1, 192]], 192),
                            ALU.max,
                        )
                        nc.vector.tensor_tensor(
                            _v(xpad2, 0, 128, [[26, 4], [1, 24]], base + yt * 4 * 26),
                            _v(xm, 0, 128, [[48, 4], [1, 24]]),
                            _v(xm, 0, 128, [[48, 4], [1, 24]], 24),
                            ALU.max,
                        )
                    else:
                        # DVE: direct 2x2 max-reduce from PSUM (relu deferred)
                        nc.vector.tensor_reduce(
                            _v(xpad2, 0, 128, [[26, 4], [1, 24]], base + yt * 4 * 26),
                            _v(ps, 0, 128, [[96, 4], [2, 24], [48, 2], [1, 2]]),
                            mybir.AxisListType.XY,
                            ALU.max,
                        )
                if 5 in yts:
                    rows = _v(xpad2, 0, 128, [[26, 8], [1, 24]], base + 16 * 26)
                    nc.vector.tensor_scalar(rows, rows, 0.0, None, ALU.max)
                # conv2 im2col: batched per chunk-PAIR (8 samples) on the gpsimd
                # SWDGE queue; quarter col layout is (g, chunk): sample
                # 16Q+4c+g at col (4g+c)*676
                Q = m // 4
                if m % 4 == 0:
                    bq = b96.tile([97, 16 * 676], BF16, tag="bq", name="bq")
                    bqs[Q] = bq
                    nc.gpsimd.dma_start(_v(bq, 96, 1, [[1, 16 * 676]]), io["ones"][0:1, 0 : 16 * 676])
                if m % 2 == 1:
                    bq = bqs[Q]
                    c0 = 2 * ((m // 2) % 2)
                    for g in range(4):
                        nc.gpsimd.dma_start(
                            _v(bq, 0, 96, [[1, 1352]], (4 * g + c0) * 676),
                            _v(xpad2, 32 * g, 32, [[1, 3], [1, 1352]], (m - 1) * 676),
                        )


            def conv2_block(b):
                bq = bqs[b // 8]
                for yh in range(2):
                    ps = c2p.tile([128, 288], F32, tag="ps2", name="ps2")
                    for h in range(2):
                        s = 2 * b + h
                        loc = 4 * (s % 4) + (s // 4 - 4 * (b // 8))
                        for dy in range(3):
                            nc.tensor.matmul(
                                _v(ps, 64 * h, 64, [[1, 288]]),
                                w2sb[0:97, dy * 64 : dy * 64 + 64],
                                _v(bq, 0, 97, [[26, 12], [1, 24]], loc * 676 + yh * 312 + dy * 26),
                                start=(dy == 0),
                                stop=(dy == 2),
                                tile_position=(0, 64 * h),
                            )
                    # Act evac (y,xh,phase), then 2x tt max stages
                    stg = c2t.tile([128, 288], BF16, tag="stg", name="stg")
                    nc.scalar.activation(
                        _v(stg, 0, 128, [[12, 12], [1, 12], [144, 2]]),
                        _v(ps, 0, 128, [[24, 12], [2, 12], [1, 2]]),
                        AF.Relu,
                    )
                    xm = c2t.tile([128, 144], BF16, tag="xm", name="xm")
                    nc.vector.tensor_tensor(
                        _v(xm, 0, 128, [[1, 144]]),
                        _v(stg, 0, 128, [[1, 144]]),
                        _v(stg, 0, 128, [[1, 144]], 144),
                        ALU.max,
                    )
                    nc.vector.tensor_tensor(
                        _v(xpad3, 0, 128, [[14, 6], [1, 12]], b * 198 + 15 + yh * 84),
                        _v(xm, 0, 128, [[24, 6], [1, 12]]),
                        _v(xm, 0, 128, [[24, 6], [1, 12]], 12),
                        ALU.max,
                    )
                # conv3 im2col chunk once its xpad3 sample range is complete
                if b == 15 or b == 31:
                    ck = b // 16
                    off = ck * 16 * 198
                    for h in range(2):
                        nc.gpsimd.dma_start(
                            _v(bufA[h], 0, 128, [[1, 16 * 198]], off),
                            _v(xpad3, 64 * h, 64, [[1, 2], [1, 16 * 198]], off),
                        )
                        nc.gpsimd.dma_start(
                            _v(bufB[h], 0, 64, [[1, 16 * 198]], off),
                            _v(xpad3, 64 * h, 64, [[1, 16 * 198]], off + 2),
                        )

            def conv3_unit(h, bp):
                # c3p/c3t are opened after the conv1 PSUM pool closes (bank budget)
                bj = bp % 4
                ps = c3p.tile([128, 288], F32, tag="ps3", name="ps3")
                for dy in range(3):
                    dims = [[198, 2], [14, 12], [1, 12]]
                    off = bp * 2 * 198 + dy * 14
                    nc.tensor.matmul(
                        ps[:, :], w3asb[0:128, dy * 128 : dy * 128 + 128],
                        _v(bufA[h], 0, 128, dims, off),
                        start=(dy == 0), stop=False,
                    )
                    nc.tensor.matmul(
                        ps[:, :], w3bsb[0:65, dy * 128 : dy * 128 + 128],
                        _v(bufB[h], 0, 65, dims, off),
                        start=False, stop=(dy == 2),
                    )
                # slot of (h, bp, i) is sample 4bp+h+2i -> feat col (4bp+h+2i)*36
                if bj < 3:
                    stg = c3t.tile([128, 288], BF16, tag="stg", name="stg")
                    nc.scalar.activation(
                        _v(stg, 0, 128, [[72, 2], [6, 12], [1, 6], [144, 2]]),
                        ps[:, :], AF.Relu,
                    )
                    xm = c3t.tile([128, 144], BF16, tag="xm", name="xm")
                    nc.vector.tensor_tensor(
                        _v(xm, 0, 128, [[1, 144]]),
                        _v(stg, 0, 128, [[1, 144]]),
                        _v(stg, 0, 128, [[1, 144]], 144),
                        ALU.max,
                    )
                    nc.vector.tensor_tensor(
                        _v(feat, 0, 128, [[72, 2], [6, 6], [1, 6]], (4 * bp + h) * 36),
                        _v(xm, 0, 128, [[72, 2], [12, 6], [1, 6]]),
                        _v(xm, 0, 128, [[72, 2], [12, 6], [1, 6]], 6),
                        ALU.max,
                    )
                else:
                    # DVE direct reduce per sample (relu deferred to feat pass)
                    for i in range(2):
                        nc.vector.tensor_reduce(
                            _v(feat, 0, 128, [[6, 6], [1, 6]], (4 * bp + h + 2 * i) * 36),
                            _v(ps, 0, 128, [[24, 6], [2, 6], [12, 2], [1, 2]], i * 144),
                            mybir.AxisListType.XY,
                            ALU.max,
                        )

            with (
                tc.tile_pool(name="c1ps", bufs=4, space="PSUM") as c1p,
                tc.tile_pool(name="c1t", bufs=3) as c1t,
            ):
                for m in range(16):
                    if m + 1 < 16:
                        imc_dma(m + 1)
                    conv1_chunk(m, c1p, c1t)
                    if m == 2:
                        late_inits()
                    if m >= 2:
                        conv2_block(2 * (m - 2))
                        conv2_block(2 * (m - 2) + 1)
            for b in range(28, 32):
                conv2_block(b)

        # ---------------- conv3 + FC1 (parity-pipelined) ----------------
        c3p = ctx.enter_context(tc.tile_pool(name="c3ps", bufs=5, space="PSUM"))
        c3t = ctx.enter_context(tc.tile_pool(name="c3t", bufs=5))
        cur1p = ctx.enter_context(tc.tile_pool(name="cur1p", bufs=1, space="PSUM"))
        cur1 = cur1p.tile([128, 256], F32)
        # ck0 units first (their im2col chunk landed at b=15); ck1's chunk
        # (emitted at b=31) and the FC1 weights transfer while these run
        for bp in range(8):
            for h in range(2):
                conv3_unit(h, bp)
            nc.gpsimd.dma_start(
                wf1sb[:, bp * 2304 : (bp + 1) * 2304],
                _dv(io["wf1"], bp * 2304, [[18432, 128], [1, 2304]]),
            )
        # s1_t straight from cur1: v=(v+c)/2, th=1, hard reset =>
        # s1_t1 = [c>=2]; s1_t2 = [c>=4/3]; s1_t3 = [c>=8/7] - [c>=4/3] + [c>=2]
        c43 = float(np.float32(4.0) / np.float32(3.0))
        c87 = float(np.float32(8.0) / np.float32(7.0))
        t87 = lifp.tile([128, 256], BF16)

        def rows_relu(h):
            # in-place relu over the DVE-reduced feat slots of this parity
            # (bp = 3,7,11,15 -> slots 4bp+h and 4bp+2+h)
            rows = _v(feat, 0, 128, [[576, 4], [72, 2], [1, 36]], (12 + h) * 36)
            nc.vector.tensor_scalar(rows, rows, 0.0, None, ALU.max)

        def fc1(h, per_g=False):
            # FC1 for parity h: out [unit, 32 samples] at cur1 col 64g+32h
            # (samples of parity h = feat cols h, h+2, ... -> stride 72)
            for g in range(4):
                for k in range(36):
                    nc.tensor.matmul(
                        cur1[:, 64 * g + 32 * h : 64 * g + 32 * h + 32],
                        wf1sb[:, k * 512 + g * 128 : k * 512 + g * 128 + 128],
                        _v(feat, 0, 128, [[72, 32]], k + 36 * h),
                        start=(k == 0),
                        stop=(k == 35),
                    )
                if per_g:
                    thr_g(h, g)

        def thresholds(h):
            # LIF layer-1 thresholds for this parity's cur1 columns
            cslc = _v(cur1, 0, 128, [[64, 4], [1, 32]], 32 * h)
            s1t = lambda t: _v(s1_all, 0, 128, [[64, 4], [1, 32]], t * 256 + 32 * h)
            nc.vector.tensor_scalar(s1t(0), cslc, 2.0, None, ALU.is_ge)
            nc.vector.tensor_scalar(s1t(1), cslc, c43, None, ALU.is_ge)
            t87s = _v(t87, 0, 128, [[64, 4], [1, 32]], 32 * h)
            nc.vector.tensor_scalar(t87s, cslc, c87, None, ALU.is_ge)
            nc.vector.tensor_tensor(s1t(2), t87s, s1t(1), ALU.subtract)
            nc.vector.tensor_tensor(s1t(2), s1t(2), s1t(0), ALU.add)

        cur2p = ctx.enter_context(tc.tile_pool(name="cur2p", bufs=1, space="PSUM"))
        dtp = ctx.enter_context(tc.tile_pool(name="liftmp", bufs=2))
        cur2 = cur2p.tile([128, 192], F32, tag="cur2")
        cur3 = cur2p.tile([5, 192], F32, tag="cur3")

        def thr_g(h, g):
            # layer-1 thresholds for one (parity, unit-group) block of cur1
            cslc = _v(cur1, 0, 128, [[1, 32]], 64 * g + 32 * h)
            s1t = lambda t: _v(s1_all, 0, 128, [[1, 32]], t * 256 + 64 * g + 32 * h)
            t87s = _v(t87, 0, 128, [[1, 32]], 64 * g + 32 * h)
            nc.vector.tensor_scalar(s1t(0), cslc, 2.0, None, ALU.is_ge)
            nc.vector.tensor_scalar(s1t(1), cslc, c43, None, ALU.is_ge)
            nc.vector.tensor_scalar(t87s, cslc, c87, None, ALU.is_ge)
            nc.vector.tensor_tensor(s1t(2), t87s, s1t(1), ALU.subtract)
            nc.vector.tensor_tensor(s1t(2), s1t(2), s1t(0), ALU.add)

        def fc2(h, ts=(0, 1, 2)):
            # FC2 for one parity over the given timesteps (wf2 pre-scaled 0.5)
            nt = len(ts)
            t0 = ts[0]
            for g in range(4):
                nc.tensor.matmul(
                    _v(cur2, 0, 128, [[64, nt], [1, 32]], 64 * t0 + 32 * h),
                    wf2sb[:, g * 128 : g * 128 + 128],
                    _v(s1_all, 0, 128, [[256, nt], [1, 32]], 256 * t0 + 64 * g + 32 * h),
                    start=(g == 0), stop=(g == 3),
                )

        def lif2(h, t):
            # v2 <- v2*0.5 + cur2_half ; s2 = (v2 >= 1) ; v2 <- 0 where s2 ; FC3
            v2h = v2[:, 32 * h : 32 * h + 32]
            s2t = _v(s2_all, 0, 128, [[1, 32]], 64 * t + 32 * h)
            nc.vector.scalar_tensor_tensor(
                v2h, v2h, 0.5, cur2[:, 64 * t + 32 * h : 64 * t + 32 * h + 32], ALU.mult, ALU.add)
            nc.vector.tensor_scalar(s2t, v2h, 1.0, None, ALU.is_ge)
            nc.vector.copy_predicated(v2h, s2t.bitcast(mybir.dt.uint16), zeros[:, 0:32])
            nc.tensor.matmul(
                cur3[0:5, 64 * t + 32 * h : 64 * t + 32 * h + 32], wf3sb[0:128, 0:5],
                _v(s2_all, 0, 128, [[1, 32]], 64 * t + 32 * h),
                start=True, stop=True,
            )

        def lif3(h, t):
            v3h = v3[0:5, 32 * h : 32 * h + 32]
            nc.vector.scalar_tensor_tensor(
                v3h, v3h, 0.5, cur3[0:5, 64 * t + 32 * h : 64 * t + 32 * h + 32], ALU.mult, ALU.add)
            s3 = dtp.tile([5, 32], F32, tag="s3")
            nc.vector.tensor_scalar(s3[:, :], v3h, 1.0, None, ALU.is_ge)
            nc.vector.copy_predicated(v3h, s3[:, :].bitcast(mybir.dt.uint32), zeros[0:5, 0:32])
            nc.vector.tensor_tensor(
                acc[0:5, 32 * h : 32 * h + 32], acc[0:5, 32 * h : 32 * h + 32], s3[:, :], ALU.add)

        def finish(h):
            # acc/3 for acc in {0,1,2,3}: mult by fp32(1/3) matches true division
            # except acc=3 (3*0.33333334 = 1.0000001) -> clamp with min(., 1.0).
            acch = acc[0:5, 32 * h : 32 * h + 32]
            nc.vector.tensor_scalar(acch, acch, float(np.float32(1.0) / np.float32(3.0)), 1.0, ALU.mult, ALU.min)
            # acc col (32h + j) holds sample 2j+h -> un-permute on the way out
            nc.sync.dma_start(
                _dv(io["out"], h, [[64, 5], [2, 32]]),
                _v(acc, 0, 5, [[1, 32]], 32 * h),
            )

        def lif_tail(h):
            fc2(h)
            lif2(h, 0)
            lif2(h, 1)
            lif3(h, 0)
            lif2(h, 2)
            lif3(h, 1)
            lif3(h, 2)

        # parity-0 ck1 units, then FC1(0) hidden behind the first parity-1
        # units; parity-0's whole LIF chain runs under conv3(1)/FC1(1)
        for bp in range(8, 16):
            conv3_unit(0, bp)
        rows_relu(0)
        for bp in range(8, 11):
            conv3_unit(1, bp)
        fc1(0)
        thresholds(0)
        fc2(0)
        conv3_unit(1, 11)
        conv3_unit(1, 12)
        lif2(0, 0)
        lif2(0, 1)
        lif2(0, 2)
        conv3_unit(1, 13)
        conv3_unit(1, 14)
        conv3_unit(1, 15)
        lif3(0, 0)
        lif3(0, 1)
        lif3(0, 2)
        finish(0)
        rows_relu(1)
        fc1(1)
        cslc1 = _v(cur1, 0, 128, [[64, 4], [1, 32]], 32)
        s1s = lambda t: _v(s1_all, 0, 128, [[64, 4], [1, 32]], t * 256 + 32)
        nc.vector.tensor_scalar(s1s(0), cslc1, 2.0, None, ALU.is_ge)
        nc.vector.tensor_scalar(s1s(1), cslc1, c43, None, ALU.is_ge)
        fc2(1, ts=(0, 1))
        t87s1 = _v(t87, 0, 128, [[64, 4], [1, 32]], 32)
        nc.vector.tensor_scalar(t87s1, cslc1, c87, None, ALU.is_ge)
        nc.vector.tensor_tensor(s1s(2), t87s1, s1s(1), ALU.subtract)
        nc.vector.tensor_tensor(s1s(2), s1s(2), s1s(0), ALU.add)
        fc2(1, ts=(2,))
        lif2(1, 0)
        lif2(1, 1)
        lif3(1, 0)
        lif2(1, 2)
        lif3(1, 1)
        lif3(1, 2)
        finish(1)


def _build():
    nc = bacc.Bacc("TRN2", target_bir_lowering=False, debug=False, enable_asserts=True)
    io = {}

    def inp(name, shape, dt):
        io[name] = nc.dram_tensor(name, shape, dt, kind="ExternalInput").ap()

    inp("imc", [109, 40000], BF16)
    inp("w1l", [109, 128], BF16)
    inp("w2l", [97, 192], BF16)
    inp("w3a", [128, 384], BF16)
    inp("w3b", [65, 384], BF16)
    inp("wf1", [128, 18432], BF16)
    inp("wf2", [128, 512], BF16)
    inp("wf3", [128, 5], BF16)
    inp("ones", [1, 10816], BF16)
    io["out"] = nc.dram_tensor("out", [5, 64], F32, kind="ExternalOutput").ap()

    with tile.TileContext(nc) as tc:
        _emit(tc, io)
    nc.compile()
    return nc


def _fake_quant(w):
    w = np.asarray(w, np.float32)
    scale = np.float32(np.max(np.abs(w)) / np.float32(127.0))
    wq = np.clip(np.round(w / scale), -127.0, 127.0).astype(np.float32) * scale
    return wq.astype(np.float32)


def _bf16(a):
    return np.asarray(a, np.float32).astype(ml_dtypes.bfloat16)


def _prep_weights(conv1_w, conv1_b, conv2_w, conv2_b, conv3_w, conv3_b, W1, W2, W3):
    c1 = np.asarray(conv1_w, np.float32)  # [32, 3, 3, 3]
    c2 = np.asarray(conv2_w, np.float32)  # [64, 32, 3, 3]
    c3 = np.asarray(conv3_w, np.float32)  # [128, 64, 3, 3]

    # conv1 block-diagonal: rows 27g..27g+26 = taps of group g -> cols 32g..32g+31;
    # row 108 = bias (tiled 4x over the 4 col groups).
    w1l = np.zeros((109, 128), np.float32)
    wk = c1.transpose(2, 3, 1, 0).reshape(27, 32)  # [(dy,dx,c), m]
    for q in range(4):
        w1l[27 * q : 27 * q + 27, 32 * q : 32 * q + 32] = wk
    w1l[108, :] = np.tile(np.asarray(conv1_b, np.float32), 4)

    w2l = np.zeros((97, 192), np.float32)
    w2l[0:96] = c2.transpose(1, 3, 2, 0).reshape(96, 192)  # [(c,dx), (dy,m)]
    w2l[96, 0:64] = np.asarray(conv2_b, np.float32)        # bias rides the dy=0 block

    w3x = c3.transpose(1, 3, 2, 0)  # [c, dx, dy, m]
    w3a = w3x[:, 0:2].reshape(128, 384)
    w3b = np.zeros((65, 384), np.float32)
    w3b[0:64] = w3x[:, 2].reshape(64, 384)
    w3b[64, 0:128] = np.asarray(conv3_b, np.float32)       # bias rides the dy=0 block

    W1q = _fake_quant(W1)  # [512, 4608]
    W2q = _fake_quant(W2)  # [128, 512]
    W3q = _fake_quant(W3)  # [5, 128]

    # [c, k*512 + u] = W1q[u, c*36 + k]  (FC1 weight-stationary: out [unit, sample])
    wf1 = W1q.reshape(512, 128, 36).transpose(1, 2, 0).reshape(128, 36 * 512)
    # FC2/FC3 pre-scaled by 0.5: LIF v-update becomes v*0.5 + cur_half in one op
    wf2 = 0.5 * W2q.T.reshape(4, 128, 128).transpose(1, 0, 2).reshape(128, 512)
    wf3 = 0.5 * W3q.T  # [128, 5]

    return {
        "w1l": _bf16(w1l),
        "w2l": _bf16(w2l),
        "w3a": _bf16(w3a),
        "w3b": _bf16(w3b),
        "wf1": _bf16(wf1),
        "wf2": _bf16(wf2),
        "wf3": _bf16(wf3),
        "ones": _bf16(np.ones((1, 10816), np.float32)),
    }


_NC = None
LAST_RESULTS = None


def kernel(x, conv1_w, conv1_b, conv2_w, conv2_b, conv3_w, conv3_b, W1, W2, W3, _trace=False):
    global _NC, LAST_RESULTS
    if _NC is None:
        _NC = _build()

    wmap = _prep_weights(conv1_w, conv1_b, conv2_w, conv2_b, conv3_w, conv3_b, W1, W2, W3)

    # host-side im2col for conv1, chunk-ordered: chunk m = samples 4m..4m+3,
    # imc[27g + dy*9 + dx*3 + c, m*2500 + p] = xpad[4m+g, c, p + dy*50 + dx]
    x = np.asarray(x, np.float32)
    xp = np.zeros((512, 3, 50, 50), np.float32)
    xp[:, :, 1:49, 1:49] = x
    xf = np.zeros((512, 3, 2604), np.float32)
    xf[:, :, :2500] = xp.reshape(512, 3, 2500)

    in_maps = []
    for i in range(NCORES):
        S = xf[B * i : B * (i + 1)].reshape(16, 4, 3, 2604)
        A = np.zeros((109, 40000), np.float32)
        for dy in range(3):
            for dx in range(3):
                blk = S[:, :, :, dy * 50 + dx : dy * 50 + dx + 2500]  # [m, g, c, p]
                for g in range(4):
                    r = 27 * g + dy * 9 + dx * 3
                    A[r : r + 3] = blk[:, g].transpose(1, 0, 2).reshape(3, 40000)
        A[108] = 1.0
        in_maps.append({"imc": _bf16(A), **wmap})

    from concourse.bass_utils import run_bass_kernel_spmd

    res = run_bass_kernel_spmd(_NC, in_maps, core_ids=list(range(NCORES)), trace=_trace)
    LAST_RESULTS = res
    out = np.concatenate([np.asarray(res.results[i]["out"]).T for i in range(NCORES)], axis=0)
    return np.ascontiguousarray(out.astype(np.float32))


# revision 57
# speedup vs baseline: 1.0096x; 1.0062x over previous
"""Trainium2 Bass kernel for ConvFCNet (3x conv+pool -> int8-fakequant FC + LIF SNN head).

Data-parallel over 8 NeuronCores: batch 512 -> 64 samples/core, weights replicated.

v1 rework (from 156us baseline): the PE queue is kept continuously fed so the
tensor engine stays at full p-state and is the binding resource (~89us of
matmul work):
  - conv1 im2col is built on the HOST in per-chunk order (chunk m = samples
    4m..4m+3 via the block-diagonal group trick), DMAed in 16 fine-grained
    chunks so the first matmul starts at ~4us instead of 11us.
  - conv2 blocks are emitted interleaved into the conv1 chunk loop (lag 2),
    so conv2 matmuls run while conv1 pooling drains instead of after it.
  - pooling max stages use tensor_tensor(max) (2x DVE perf mode for packed
    bf16) instead of scalar_tensor_tensor (no perf mode), and PSUM tiles span
    2 banks so one Act evacuation covers 2 matmul tiles.
  - LIF layer-1 is solved analytically across the 3 timesteps straight from
    the cur1 PSUM (s1_t thresholds 2, 4/3, 8/7 on cur1), FC2 runs all 3
    timesteps in one matmul set (N=192), and FC2/FC3 weights are pre-scaled
    by 0.5 on the host so the LIF v-update is a single scalar_tensor_tensor.
"""

import numpy as np
import ml_dtypes

import concourse.bass as bass
import concourse.bacc as bacc
import concourse.tile as tile
import concourse.mybir as mybir

AF = mybir.ActivationFunctionType
ALU = mybir.AluOpType
BF16 = mybir.dt.bfloat16
F32 = mybir.dt.float32

NCORES = 8
B = 64  # samples per core


def _v(ap, p0, npart, dims, off=0):
    """View into an SBUF/PSUM tile AP: partition slice [p0, p0+npart) + custom free dims."""
    pitch = ap.ap[0][0]
    return bass.AP(
        tensor=ap.tensor,
        offset=ap.offset + p0 * pitch + off,
        ap=[[pitch, npart]] + [list(d) for d in dims],
    )


def _dv(ap, off, dims):
    """View into a DRAM tensor AP with custom dims."""
    return bass.AP(tensor=ap.tensor, offset=ap.offset + off, ap=[list(d) for d in dims])


def _emit(tc, io):
    nc = tc.nc
    from contextlib import ExitStack

    with ExitStack() as ctx:
        # ---------------- persistent buffers + weights ----------------
        # weights ride the Activation HWDGE queue: Pool stays free for buf96 descriptor
        # generation and SP for the im2col stream
        c1i = ctx.enter_context(tc.tile_pool(name="c1imc", bufs=5))
        imct0 = c1i.tile([109, 2400], BF16, tag="imc", name="imc")
        nc.sync.dma_start(
            _v(imct0, 0, 109, [[1, 800]]),
            _dv(io["imc"], 0, [[40000, 109], [1, 800]]),
        )
        nc.sync.dma_start(
            _v(imct0, 0, 109, [[1, 1600]], 800),
            _dv(io["imc"], 800, [[40000, 109], [1, 1600]]),
        )
        wp = ctx.enter_context(tc.tile_pool(name="wts", bufs=1))
        w1sb = wp.tile([109, 128], BF16)
        nc.scalar.dma_start(w1sb[:, :], io["w1l"][:, :])
        w2sb = wp.tile([97, 192], BF16)
        nc.scalar.dma_start(w2sb[:, :], io["w2l"][:, :])
        w3asb = wp.tile([128, 384], BF16)
        w3bsb = wp.tile([65, 384], BF16)
        wf2sb = wp.tile([128, 512], BF16)
        wf3sb = wp.tile([128, 5], BF16)

        # preload the Relu activation table while the head DMAs run
        scr = wp.tile([1, 8], BF16)
        nc.scalar.activation(_v(scr, 0, 1, [[1, 8]]), _v(w1sb, 0, 1, [[1, 8]]), AF.Relu)

        mp = ctx.enter_context(tc.tile_pool(name="main", bufs=1))
        # conv1 pooled output, padded 26x26; partition 32g+c = sample 4m+g at col m*676
        xpad2 = mp.tile([128, 16 * 676 + 4], BF16)
        for dims, off in [
            ([[676, 16], [1, 26]], 0),        # top row
            ([[676, 16], [1, 26]], 650),      # bottom row
            ([[676, 16], [26, 26]], 0),       # left col
            ([[676, 16], [26, 26]], 25),      # right col
            ([[1, 4]], 16 * 676),             # tail pad (im2col dx over-read)
        ]:
            nc.gpsimd.memset(_v(xpad2, 0, 128, dims, off), 0.0)
        # conv2 pooled output, padded 14x14, partition 64h+c holds samples of parity h
        xpad3 = mp.tile([128, 32 * 198 + 4], BF16)
        # conv3 pooled output (features): [128c, sample*36 + hw]
        feat = mp.tile([128, B * 36], BF16)

        # LIF state
        lifp = ctx.enter_context(tc.tile_pool(name="lif", bufs=1))
        zeros = lifp.tile([128, 64], F32)
        v2 = lifp.tile([128, 64], F32)
        v3 = lifp.tile([5, 64], F32)
        acc = lifp.tile([5, 64], F32)
        s1_all = lifp.tile([128, 768], BF16)   # [t*256 + cur1-col]
        s2_all = lifp.tile([128, 192], BF16)   # [t*64 + sample-col]

        # conv3 im2col buffers (row 64 of B = bias row)
        c3b = ctx.enter_context(tc.tile_pool(name="c3buf", bufs=1))
        bufA = [c3b.tile([128, 32 * 198 + 4], BF16, name=f"bufA{h}") for h in range(2)]
        bufB = [c3b.tile([65, 32 * 198 + 4], BF16, name=f"bufB{h}") for h in range(2)]

        def late_inits():
            # not needed until conv2/the tail: emitted on the gpsimd queue after
            # the first buf96 pair DMAs so they don't delay the conv2 start
            for dims, off in [
                ([[198, 32], [1, 14]], 0),        # top row
                ([[198, 32], [1, 14]], 182),      # bottom row
                ([[198, 32], [14, 14]], 0),       # left col
                ([[198, 32], [14, 14]], 13),      # right col
                ([[1, 4]], 32 * 198),             # tail pad (im2col dx over-read)
                ([[198, 32], [1, 2]], 196),       # per-sample slack (pitch 198 vs 196)
            ]:
                nc.gpsimd.memset(_v(xpad3, 0, 128, dims, off), 0.0)
            for t in (zeros, v2, v3, acc):
                nc.gpsimd.memset(t[:, :], 0.0)
            nc.gpsimd.dma_start(w3asb[:, :], io["w3a"][:, :])
            nc.gpsimd.dma_start(w3bsb[:, :], io["w3b"][:, :])
            nc.gpsimd.dma_start(wf2sb[:, :], io["wf2"][:, :])
            nc.gpsimd.dma_start(wf3sb[:, :], io["wf3"][:, :])
            for h in range(2):
                nc.gpsimd.dma_start(_v(bufB[h], 64, 1, [[1, 32 * 198 + 4]]), io["ones"][0:1, 0 : 32 * 198 + 4])

        # FC1 weights: loaded in 4 chunks spread across the conv1/conv2 window
        # (a single 13us DMA would block the serialized DMA engines)
        fcw = ctx.enter_context(tc.tile_pool(name="fcw", bufs=1))
        wf1sb = fcw.tile([128, 18432], BF16)

        # conv2 im2col quarters (96 rows = 32c x 3dx, row 96 = bias row), scoped
        b96 = ctx.enter_context(tc.tile_pool(name="b96", bufs=2))
        bqs = {}

        # ---------------- conv1 + conv2 + conv3 (interleaved, PE stays fed) ----------------
        with (
            tc.tile_pool(name="c2ps", bufs=4, space="PSUM") as c2p,
            tc.tile_pool(name="c2t", bufs=3) as c2t,
        ):
            imcts = {0: imct0}

            def imc_dma(m):
                imct = c1i.tile([109, 2400], BF16, tag="imc", name="imc")
                nc.sync.dma_start(
                    _v(imct, 0, 109, [[1, 2400]]),
                    _dv(io["imc"], m * 2500, [[40000, 109], [1, 2400]]),
                )
                imcts[m] = imct

            def conv1_chunk(m, c1p, c1t, yts=range(6)):
                imct = imcts[m]
                base = m * 676 + 27
                for yt in yts:
                    ps = c1p.tile([128, 384], F32, tag="ps1", name="ps1")
                    nc.tensor.matmul(
                        ps[:, :],
                        _v(w1sb, 0, 109, [[1, 128]]),
                        _v(imct, 0, 109, [[50, 8], [1, 48]], yt * 400),
                        start=True,
                        stop=True,
                    )
                    if yt < 4:
                        # Act evac: relu+copy, x-deinterleaved (y,xh,phase)
                        stg = c1t.tile([128, 384], BF16, tag="stg", name="stg")
                        nc.scalar.activation(
                            _v(stg, 0, 128, [[24, 8], [1, 24], [192, 2]]),
                            ps[:, :],
                            AF.Relu,
                        )
                        # max stages as tensor_tensor (2x DVE mode on packed bf16)
                        xm = c1t.tile([128, 192], BF16, tag="xm", name="xm")
                        nc.vector.tensor_tensor(
                            _v(xm, 0, 128, [[1, 192]]),
                            _v(stg, 0, 128, [[1, 192]]),
                            _v(stg, 0, 128, [[1, 192]], 192),
                            ALU.max,
                        )
                        nc.vector.tensor_tensor(
                            _v(xpad2, 0, 128, [[26, 4], [1, 24]], base + yt * 4 * 26),
                            _v(xm, 0, 128, [[48, 4], [1, 24]]),
                            _v(xm, 0, 128, [[48, 4], [1, 24]], 24),
                            ALU.max,
                        )
                    else:
                        # DVE: direct 2x2 max-reduce from PSUM (relu deferred)
                        nc.vector.tensor_reduce(
                            _v(xpad2, 0, 128, [[26, 4], [1, 24]], base + yt * 4 * 26),
                            _v(ps, 0, 128, [[96, 4], [2, 24], [48, 2], [1, 2]]),
                            mybir.AxisListType.XY,
                            ALU.max,
                        )
                if 5 in yts:
                    rows = _v(xpad2, 0, 128, [[26, 8], [1, 24]], base + 16 * 26)
                    nc.vector.tensor_scalar(rows, rows, 0.0, None, ALU.max)
                # conv2 im2col: batched per chunk-PAIR (8 samples) on the gpsimd
                # SWDGE queue; quarter col layout is (g, chunk): sample
                # 16Q+4c+g at col (4g+c)*676
                Q = m // 4
                if m % 4 == 0:
                    bq = b96.tile([97, 16 * 676], BF16, tag="bq", name="bq")
                    bqs[Q] = bq
                    nc.gpsimd.dma_start(_v(bq, 96, 1, [[1, 16 * 676]]), io["ones"][0:1, 0 : 16 * 676])
                if m % 2 == 1:
                    bq = bqs[Q]
                    c0 = 2 * ((m // 2) % 2)
                    for g in range(4):
                        nc.gpsimd.dma_start(
                            _v(bq, 0, 96, [[1, 1352]], (4 * g + c0) * 676),
                            _v(xpad2, 32 * g, 32, [[1, 3], [1, 1352]], (m - 1) * 676),
                        )


            def conv2_block(b):
                bq = bqs[b // 8]
                for yh in range(2):
                    ps = c2p.tile([128, 288], F32, tag="ps2", name="ps2")
                    for h in range(2):
                        s = 2 * b + h
                        loc = 4 * (s % 4) + (s // 4 - 4 * (b // 8))
                        for dy in range(3):
                            nc.tensor.matmul(
                                _v(ps, 64 * h, 64, [[1, 288]]),
                                w2sb[0:97, dy * 64 : dy * 64 + 64],
                                _v(bq, 0, 97, [[26, 12], [1, 24]], loc * 676 + yh * 312 + dy * 26),
                                start=(dy == 0),
                                stop=(dy == 2),
                                tile_position=(0, 64 * h),
                            )
                    # Act evac (y,xh,phase), then 2x tt max stages
                    stg = c2t.tile([128, 288], BF16, tag="stg", name="stg")
                    nc.scalar.activation(
                        _v(stg, 0, 128, [[12, 12], [1, 12], [144, 2]]),
                        _v(ps, 0, 128, [[24, 12], [2, 12], [1, 2]]),
                        AF.Relu,
                    )
                    xm = c2t.tile([128, 144], BF16, tag="xm", name="xm")
                    nc.vector.tensor_tensor(
                        _v(xm, 0, 128, [[1, 144]]),
                        _v(stg, 0, 128, [[1, 144]]),
                        _v(stg, 0, 128, [[1, 144]], 144),
                        ALU.max,
                    )
                    nc.vector.tensor_tensor(
                        _v(xpad3, 0, 128, [[14, 6], [1, 12]], b * 198 + 15 + yh * 84),
                        _v(xm, 0, 128, [[24, 6], [1, 12]]),
                        _v(xm, 0, 128, [[24, 6], [1, 12]], 12),
                        ALU.max,
                    )
                # conv3 im2col chunk once its xpad3 sample range is complete
                if b == 15 or b == 31:
                    ck = b // 16
                    off = ck * 16 * 198
                    for h in range(2):
                        nc.gpsimd.dma_start(
                            _v(bufA[h], 0, 128, [[1, 16 * 198]], off),
                            _v(xpad3, 64 * h, 64, [[1, 2], [1, 16 * 198]], off),
                        )
                        nc.gpsimd.dma_start(
                            _v(bufB[h], 0, 64, [[1, 16 * 198]], off),
                            _v(xpad3, 64 * h, 64, [[1, 16 * 198]], off + 2),
                        )

            def conv3_unit(h, bp):
                # c3p/c3t are opened after the conv1 PSUM pool closes (bank budget)
                bj = bp % 4
                ps = c3p.tile([128, 288], F32, tag="ps3", name="ps3")
                for dy in range(3):
                    dims = [[198, 2], [14, 12], [1, 12]]
                    off = bp * 2 * 198 + dy * 14
                    nc.tensor.matmul(
                        ps[:, :], w3asb[0:128, dy * 128 : dy * 128 + 128],
                        _v(bufA[h], 0, 128, dims, off),
                        start=(dy == 0), stop=False,
                    )
                    nc.tensor.matmul(
                        ps[:, :], w3bsb[0:65, dy * 128 : dy * 128 + 128],
                        _v(bufB[h], 0, 65, dims, off),
                        start=False, stop=(dy == 2),
                    )
                # slot of (h, bp, i) is sample 4bp+h+2i -> feat col (4bp+h+2i)*36
                if bj < 3:
                    stg = c3t.tile([128, 288], BF16, tag="stg", name="stg")
                    nc.scalar.activation(
                        _v(stg, 0, 128, [[72, 2], [6, 12], [1, 6], [144, 2]]),
                        ps[:, :], AF.Relu,
                    )
                    xm = c3t.tile([128, 144], BF16, tag="xm", name="xm")
                    nc.vector.tensor_tensor(
                        _v(xm, 0, 128, [[1, 144]]),
                        _v(stg, 0, 128, [[1, 144]]),
                        _v(stg, 0, 128, [[1, 144]], 144),
                        ALU.max,
                    )
                    nc.vector.tensor_tensor(
                        _v(feat, 0, 128, [[72, 2], [6, 6], [1, 6]], (4 * bp + h) * 36),
                        _v(xm, 0, 128, [[72, 2], [12, 6], [1, 6]]),
                        _v(xm, 0, 128, [[72, 2], [12, 6], [1, 6]], 6),
                        ALU.max,
                    )
                else:
                    # DVE direct reduce per sample (relu deferred to feat pass)
                    for i in range(2):
                        nc.vector.tensor_reduce(
                            _v(feat, 0, 128, [[6, 6], [1, 6]], (4 * bp + h + 2 * i) * 36),
                            _v(ps, 0, 128, [[24, 6], [2, 6], [12, 2], [1, 2]], i * 144),
                            mybir.AxisListType.XY,
                            ALU.max,
                        )

            with (
                tc.tile_pool(name="c1ps", bufs=4, space="PSUM") as c1p,
                tc.tile_pool(name="c1t", bufs=3) as c1t,
            ):
                for m in range(16):
                    if m + 1 < 16:
                        imc_dma(m + 1)
                    conv1_chunk(m, c1p, c1t)
                    if m == 2:
                        late_inits()
                    if m >= 2:
                        conv2_block(2 * (m - 2))
                        conv2_block(2 * (m - 2) + 1)
            for b in range(28, 32):
                conv2_block(b)

        # ---------------- conv3 + FC1 (parity-pipelined) ----------------
        c3p = ctx.enter_context(tc.tile_pool(name="c3ps", bufs=5, space="PSUM"))
        c3t = ctx.enter_context(tc.tile_pool(name="c3t", bufs=5))
        cur1p = ctx.enter_context(tc.tile_pool(name="cur1p", bufs=1, space="PSUM"))
        cur1 = cur1p.tile([128, 256], F32)
        # ck0 units first (their im2col chunk landed at b=15); ck1's chunk
        # (emitted at b=31) and the FC1 weights transfer while these run
        for bp in range(8):
            for h in range(2):
                conv3_unit(h, bp)
            nc.gpsimd.dma_start(
                wf1sb[:, bp * 2304 : (bp + 1) * 2304],
                _dv(io["wf1"], bp * 2304, [[18432, 128], [1, 2304]]),
            )
        # s1_t straight from cur1: v=(v+c)/2, th=1, hard reset =>
        # s1_t1 = [c>=2]; s1_t2 = [c>=4/3]; s1_t3 = [c>=8/7] - [c>=4/3] + [c>=2]
        c43 = float(np.float32(4.0) / np.float32(3.0))
        c87 = float(np.float32(8.0) / np.float32(7.0))
        t87 = lifp.tile([128, 256], BF16)

        def rows_relu(h):
            # in-place relu over the DVE-reduced feat slots of this parity
            # (bp = 3,7,11,15 -> slots 4bp+h and 4bp+2+h)
            rows = _v(feat, 0, 128, [[576, 4], [72, 2], [1, 36]], (12 + h) * 36)
            nc.vector.tensor_scalar(rows, rows, 0.0, None, ALU.max)

        def fc1(h, per_g=False):
            # FC1 for parity h: out [unit, 32 samples] at cur1 col 64g+32h
            # (samples of parity h = feat cols h, h+2, ... -> stride 72)
            for g in range(4):
                for k in range(36):
                    nc.tensor.matmul(
                        cur1[:, 64 * g + 32 * h : 64 * g + 32 * h + 32],
                        wf1sb[:, k * 512 + g * 128 : k * 512 + g * 128 + 128],
                        _v(feat, 0, 128, [[72, 32]], k + 36 * h),
                        start=(k == 0),
                        stop=(k == 35),
                    )
                if per_g:
                    thr_g(h, g)

        def thresholds(h):
            # LIF layer-1 thresholds for this parity's cur1 columns
            cslc = _v(cur1, 0, 128, [[64, 4], [1, 32]], 32 * h)
            s1t = lambda t: _v(s1_all, 0, 128, [[64, 4], [1, 32]], t * 256 + 32 * h)
            nc.vector.tensor_scalar(s1t(0), cslc, 2.0, None, ALU.is_ge)
            nc.vector.tensor_scalar(s1t(1), cslc, c43, None, ALU.is_ge)
            t87s = _v(t87, 0, 128, [[64, 4], [1, 32]], 32 * h)
            nc.vector.tensor_scalar(t87s, cslc, c87, None, ALU.is_ge)
            nc.vector.tensor_tensor(s1t(2), t87s, s1t(1), ALU.subtract)
            nc.vector.tensor_tensor(s1t(2), s1t(2), s1t(0), ALU.add)

        cur2p = ctx.enter_context(tc.tile_pool(name="cur2p", bufs=1, space="PSUM"))
        dtp = ctx.enter_context(tc.tile_pool(name="liftmp", bufs=2))
        cur2 = cur2p.tile([128, 192], F32, tag="cur2")
        cur3 = cur2p.tile([5, 192], F32, tag="cur3")

        def thr_g(h, g):
            # layer-1 thresholds for one (parity, unit-group) block of cur1
            cslc = _v(cur1, 0, 128, [[1, 32]], 64 * g + 32 * h)
            s1t = lambda t: _v(s1_all, 0, 128, [[1, 32]], t * 256 + 64 * g + 32 * h)
            t87s = _v(t87, 0, 128, [[1, 32]], 64 * g + 32 * h)
            nc.vector.tensor_scalar(s1t(0), cslc, 2.0, None, ALU.is_ge)
            nc.vector.tensor_scalar(s1t(1), cslc, c43, None, ALU.is_ge)
            nc.vector.tensor_scalar(t87s, cslc, c87, None, ALU.is_ge)
            nc.vector.tensor_tensor(s1t(2), t87s, s1t(1), ALU.subtract)
            nc.vector.tensor_tensor(s1t(2), s1t(2), s1t(0), ALU.add)

        def fc2(h, ts=(0, 1, 2)):
            # FC2 for one parity over the given timesteps (wf2 pre-scaled 0.5)
            nt = len(ts)
            t0 = ts[0]
            for g in range(4):
                nc.tensor.matmul(
                    _v(cur2, 0, 128, [[64, nt], [1, 32]], 64 * t0 + 32 * h),
                    wf2sb[:, g * 128 : g * 128 + 128],
                    _v(s1_all, 0, 128, [[256, nt], [1, 32]], 256 * t0 + 64 * g + 32 * h),
                    start=(g == 0), stop=(g == 3),
                )

        def lif2(h, t):
            # v2 <- v2*0.5 + cur2_half ; s2 = (v2 >= 1) ; v2 <- 0 where s2 ; FC3
            v2h = v2[:, 32 * h : 32 * h + 32]
            s2t = _v(s2_all, 0, 128, [[1, 32]], 64 * t + 32 * h)
            nc.vector.scalar_tensor_tensor(
                v2h, v2h, 0.5, cur2[:, 64 * t + 32 * h : 64 * t + 32 * h + 32], ALU.mult, ALU.add)
            nc.vector.tensor_scalar(s2t, v2h, 1.0, None, ALU.is_ge)
            nc.vector.copy_predicated(v2h, s2t.bitcast(mybir.dt.uint16), zeros[:, 0:32])
            nc.tensor.matmul(
                cur3[0:5, 64 * t + 32 * h : 64 * t + 32 * h + 32], wf3sb[0:128, 0:5],
                _v(s2_all, 0, 128, [[1, 32]], 64 * t + 32 * h),
                start=True, stop=True,
            )

        def lif3(h, t):
            v3h = v3[0:5, 32 * h : 32 * h + 32]
            nc.vector.scalar_tensor_tensor(
                v3h, v3h, 0.5, cur3[0:5, 64 * t + 32 * h : 64 * t + 32 * h + 32], ALU.mult, ALU.add)
            s3 = dtp.tile([5, 32], F32, tag="s3")
            nc.vector.tensor_scalar(s3[:, :], v3h, 1.0, None, ALU.is_ge)
            nc.vector.copy_predicated(v3h, s3[:, :].bitcast(mybir.dt.uint32), zeros[0:5, 0:32])
            nc.vector.tensor_tensor(
                acc[0:5, 32 * h : 32 * h + 32], acc[0:5, 32 * h : 32 * h + 32], s3[:, :], ALU.add)

        def finish(h):
            # acc/3 for acc in {0,1,2,3}: mult by fp32(1/3) matches true division
            # except acc=3 (3*0.33333334 = 1.0000001) -> clamp with min(., 1.0).
            acch = acc[0:5, 32 * h : 32 * h + 32]
            nc.vector.tensor_scalar(acch, acch, float(np.float32(1.0) / np.float32(3.0)), 1.0, ALU.mult, ALU.min)
            # acc col (32h + j) holds sample 2j+h -> un-permute on the way out
            nc.sync.dma_start(
                _dv(io["out"], h, [[64, 5], [2, 32]]),
                _v(acc, 0, 5, [[1, 32]], 32 * h),
            )

        def lif_tail(h):
            fc2(h)
            lif2(h, 0)
            lif2(h, 1)
            lif3(h, 0)
            lif2(h, 2)
            lif3(h, 1)
            lif3(h, 2)

        # parity-0 ck1 units, then FC1(0) hidden behind the first parity-1
        # units; parity-0's whole LIF chain runs under conv3(1)/FC1(1)
        for bp in range(8, 16):
            conv3_unit(0, bp)
        rows_relu(0)
        for bp in range(8, 11):
            conv3_unit(1, bp)
        fc1(0)
        thresholds(0)
        fc2(0)
        conv3_unit(1, 11)
        conv3_unit(1, 12)
        lif2(0, 0)
        lif2(0, 1)
        lif2(0, 2)
        conv3_unit(1, 13)
        conv3_unit(1, 14)
        conv3_unit(1, 15)
        lif3(0, 0)
        lif3(0, 1)
        lif3(0, 2)
        finish(0)
        rows_relu(1)
        fc1(1)
        cslc1 = _v(cur1, 0, 128, [[64, 4], [1, 32]], 32)
        s1s = lambda t: _v(s1_all, 0, 128, [[64, 4], [1, 32]], t * 256 + 32)
        nc.vector.tensor_scalar(s1s(0), cslc1, 2.0, None, ALU.is_ge)
        nc.vector.tensor_scalar(s1s(1), cslc1, c43, None, ALU.is_ge)
        fc2(1, ts=(0, 1))
        t87s1 = _v(t87, 0, 128, [[64, 4], [1, 32]], 32)
        nc.vector.tensor_scalar(t87s1, cslc1, c87, None, ALU.is_ge)
        nc.vector.tensor_tensor(s1s(2), t87s1, s1s(1), ALU.subtract)
        nc.vector.tensor_tensor(s1s(2), s1s(2), s1s(0), ALU.add)
        fc2(1, ts=(2,))
        lif2(1, 0)
        lif2(1, 1)
        lif3(1, 0)
        lif2(1, 2)
        lif3(1, 1)
        lif3(1, 2)
        finish(1)


def _build():
    nc = bacc.Bacc("TRN2", target_bir_lowering=False, debug=False, enable_asserts=True)
    io = {}

    def inp(name, shape, dt):
        io[name] = nc.dram_tensor(name, shape, dt, kind="ExternalInput").ap()

    inp("imc", [109, 40000], BF16)
    inp("w1l", [109, 128], BF16)
    inp("w2l", [97, 192], BF16)
    inp("w3a", [128, 384], BF16)
    inp("w3b", [65, 384], BF16)
    inp("wf1", [128, 18432], BF16)
    inp("wf2", [128, 512], BF16)
    inp("wf3", [128, 5], BF16)
    inp("ones", [1, 10816], BF16)
    io["out"] = nc.dram_tensor("out", [5, 64], F32, kind="ExternalOutput").ap()

    with tile.TileContext(nc) as tc:
        _emit(tc, io)
    nc.compile()
    return nc


def _fake_quant(w):
    w = np.asarray(w, np.float32)
    scale = np.float32(np.max(np.abs(w)) / np.float32(127.0))
    wq = np.clip(np.round(w / scale), -127.0, 127.0).astype(np.float32) * scale
    return wq.astype(np.float32)


def _bf16(a):
    return np.asarray(a, np.float32).astype(ml_dtypes.bfloat16)


def _prep_weights(conv1_w, conv1_b, conv2_w, conv2_b, conv3_w, conv3_b, W1, W2, W3):
    c1 = np.asarray(conv1_w, np.float32)  # [32, 3, 3, 3]
    c2 = np.asarray(conv2_w, np.float32)  # [64, 32, 3, 3]
    c3 = np.asarray(conv3_w, np.float32)  # [128, 64, 3, 3]

    # conv1 block-diagonal: rows 27g..27g+26 = taps of group g -> cols 32g..32g+31;
    # row 108 = bias (tiled 4x over the 4 col groups).
    w1l = np.zeros((109, 128), np.float32)
    wk = c1.transpose(2, 3, 1, 0).reshape(27, 32)  # [(dy,dx,c), m]
    for q in range(4):
        w1l[27 * q : 27 * q + 27, 32 * q : 32 * q + 32] = wk
    w1l[108, :] = np.tile(np.asarray(conv1_b, np.float32), 4)

    w2l = np.zeros((97, 192), np.float32)
    w2l[0:96] = c2.transpose(1, 3, 2, 0).reshape(96, 192)  # [(c,dx), (dy,m)]
    w2l[96, 0:64] = np.asarray(conv2_b, np.float32)        # bias rides the dy=0 block

    w3x = c3.transpose(1, 3, 2, 0)  # [c, dx, dy, m]
    w3a = w3x[:, 0:2].reshape(128, 384)
    w3b = np.zeros((65, 384), np.float32)
    w3b[0:64] = w3x[:, 2].reshape(64, 384)
    w3b[64, 0:128] = np.asarray(conv3_b, np.float32)       # bias rides the dy=0 block

    W1q = _fake_quant(W1)  # [512, 4608]
    W2q = _fake_quant(W2)  # [128, 512]
    W3q = _fake_quant(W3)  # [5, 128]

    # [c, k*512 + u] = W1q[u, c*36 + k]  (FC1 weight-stationary: out [unit, sample])
    wf1 = W1q.reshape(512, 128, 36).transpose(1, 2, 0).reshape(128, 36 * 512)
    # FC2/FC3 pre-scaled by 0.5: LIF v-update becomes v*0.5 + cur_half in one op
    wf2 = 0.5 * W2q.T.reshape(4, 128, 128).transpose(1, 0, 2).reshape(128, 512)
    wf3 = 0.5 * W3q.T  # [128, 5]

    return {
        "w1l": _bf16(w1l),
        "w2l": _bf16(w2l),
        "w3a": _bf16(w3a),
        "w3b": _bf16(w3b),
        "wf1": _bf16(wf1),
        "wf2": _bf16(wf2),
        "wf3": _bf16(wf3),
        "ones": _bf16(np.ones((1, 10816), np.float32)),
    }


_NC = None
LAST_RESULTS = None


def kernel(x, conv1_w, conv1_b, conv2_w, conv2_b, conv3_w, conv3_b, W1, W2, W3, _trace=False):
    global _NC, LAST_RESULTS
    if _NC is None:
        _NC = _build()

    wmap = _prep_weights(conv1_w, conv1_b, conv2_w, conv2_b, conv3_w, conv3_b, W1, W2, W3)

    # host-side im2col for conv1, chunk-ordered: chunk m = samples 4m..4m+3,
    # imc[27g + dy*9 + dx*3 + c, m*2500 + p] = xpad[4m+g, c, p + dy*50 + dx]
    x = np.asarray(x, np.float32)
    xp = np.zeros((512, 3, 50, 50), np.float32)
    xp[:, :, 1:49, 1:49] = x
    xf = np.zeros((512, 3, 2604), np.float32)
    xf[:, :, :2500] = xp.reshape(512, 3, 2500)

    in_maps = []
    for i in range(NCORES):
        S = xf[B * i : B * (i + 1)].reshape(16, 4, 3, 2604)
        A = np.zeros((109, 40000), np.float32)
        for dy in range(3):
            for dx in range(3):
                blk = S[:, :, :, dy * 50 + dx : dy * 50 + dx + 2500]  # [m, g, c, p]
                for g in range(4):
                    r = 27 * g + dy * 9 + dx * 3
                    A[r : r + 3] = blk[:, g].transpose(1, 0, 2).reshape(3, 40000)
        A[108] = 1.0
        in_maps.append({"imc": _bf16(A), **wmap})

    from concourse.bass_utils import run_bass_kernel_spmd

    res = run_bass_kernel_spmd(_NC, in_maps, core_ids=list(range(NCORES)), trace=_trace)
    LAST_RESULTS = res
    out = np.concatenate([np.asarray(res.results[i]["out"]).T for i in range(NCORES)], axis=0)
    return np.ascontiguousarray(out.astype(np.float32))
